# revision 2
# baseline (speedup 1.0000x reference)
"""LinearAttention Trainium2 Bass kernel.

Full-input contract: kernel(**inputs) takes the unsharded inputs from
setup_inputs() and returns the full output. Internally shards data-parallel
over batch (b=8) across 8 NeuronCores; each core computes one batch item's
full linear attention:

  qkv = w_qkv @ x        (1x1 conv, layout B on chip: [n, 768], n on partitions)
  q = softmax_d(q); k = softmax_n(k)
  ctx[h] = k_h @ v_h^T   (accumulated over n in PSUM, fp32r matmuls)
  att[h] = ctx[h]^T @ q  (after PE-transposing normalized exp(q) to layout A)
  out = w_out @ att + b_out

All matmuls run in float32r (full PE rate at moving-dim >= 256).
"""

import numpy as np

import concourse.bass as bass
import concourse.tile as tile
from concourse import bacc, mybir
from concourse.bass_utils import run_bass_kernel_spmd
from concourse.masks import make_identity

F32 = mybir.dt.float32
F32R = mybir.dt.float32r
AF = mybir.ActivationFunctionType

C = 128          # input channels
N = 16384        # h*w
HEADS = 4
DH = 64          # dim_head
INNER = HEADS * DH          # 256
QKV = 3 * INNER             # 768
NB = 512                    # pass block width (columns of n)
SUB = NB // 128             # 128-col subtiles per block
NBLK = N // NB              # 32
NSUB = N // 128             # 128


def _r(ap):
    return ap.bitcast(F32R)


def build_nc():
    nc = bacc.Bacc("TRN2", target_bir_lowering=False, debug=False, num_devices=8)

    x = nc.dram_tensor("x", [C, N], F32R, kind="ExternalInput")
    wqT = nc.dram_tensor("wqT", [C, QKV], F32R, kind="ExternalInput")
    woT = nc.dram_tensor("woT", [INNER, C], F32R, kind="ExternalInput")
    bo = nc.dram_tensor("bo", [C, 1], F32, kind="ExternalInput")
    out = nc.dram_tensor("out", [C, N], F32, kind="ExternalOutput")

    with tile.TileContext(nc) as tc:
        with (
            tc.tile_pool(name="consts", bufs=1) as consts,
            tc.tile_pool(name="eqa", bufs=1) as eqa,
            tc.tile_pool(name="xin", bufs=3) as xin,
            tc.tile_pool(name="work", bufs=3) as work,
            tc.tile_pool(name="small", bufs=4) as small,
        ):
            # ---- constants ----
            wq_s = consts.tile([C, QKV], F32R)
            nc.sync.dma_start(out=wq_s, in_=wqT[:, :])
            wo_s = consts.tile([C, 2, C], F32R)
            nc.sync.dma_start(out=wo_s[:, 0, :], in_=woT[0:128, :])
            nc.sync.dma_start(out=wo_s[:, 1, :], in_=woT[128:256, :])
            bo_s = consts.tile([C, 1], F32)
            nc.sync.dma_start(out=bo_s, in_=bo[:, :])
            ident = consts.tile([C, C], F32)
            make_identity(nc, ident)
            ident_r = consts.tile([C, C], F32R)
            nc.vector.tensor_copy(ident_r, ident)

            # resident transposed normalized exp(q), layout A (d-pack on partitions)
            eqnA01 = eqa.tile([C, N], F32R)
            eqnA23 = eqa.tile([C, N], F32R)

            # ---- pass 1 ----
            with (
                tc.tile_pool(name="qkvp", bufs=2, space="PSUM") as qkvp,
                tc.tile_pool(name="ctxp", bufs=1, space="PSUM") as ctxp,
                tc.tile_pool(name="trp", bufs=2, space="PSUM") as trp,
            ):
                ctx01 = ctxp.tile([C, INNER + 2], F32)
                ctx23 = ctxp.tile([C, INNER + 2], F32)

                x_blk = None
                for t in range(NSUB):
                    blk, s = divmod(t, SUB)
                    if s == 0:
                        x_blk = xin.tile([C, NB], F32R, tag="x_blk")
                        nc.sync.dma_start(
                            out=x_blk, in_=x[:, blk * NB : (blk + 1) * NB]
                        )
                    xs = x_blk[:, s * 128 : (s + 1) * 128]

                    qkv = qkvp.tile([C, QKV], F32, tag="qkv")
                    nc.tensor.matmul(
                        qkv[:, 0:512], lhsT=xs, rhs=wq_s[:, 0:512],
                        start=True, stop=True, skip_group_check=True,
                    )
                    nc.tensor.matmul(
                        qkv[:, 512:768], lhsT=xs, rhs=wq_s[:, 512:768],
                        start=True, stop=True, skip_group_check=True,
                    )

                    # q: exp, per-head sum, reciprocal, scale (layout B)
                    eq = work.tile([C, HEADS, DH], F32, tag="eq")
                    nc.scalar.activation(eq[:, :, :], qkv[:, 0:256], AF.Exp)
                    sq = small.tile([C, HEADS], F32, tag="sq")
                    nc.vector.reduce_sum(sq, eq[:, :, :], axis=mybir.AxisListType.X)
                    rq = small.tile([C, HEADS], F32, tag="rq")
                    nc.vector.reciprocal(rq, sq)
                    eqn = work.tile([C, HEADS, DH], F32R, tag="eqn")
                    for h in range(HEADS):
                        nc.vector.tensor_scalar_mul(
                            eqn[:, h, :], eq[:, h, :], rq[:, h : h + 1]
                        )

                    # k, v
                    ek = work.tile([C, INNER], F32R, tag="ek")
                    nc.scalar.activation(ek, qkv[:, 256:512], AF.Exp)
                    vt = work.tile([C, INNER + 2], F32R, tag="vt")
                    nc.vector.tensor_copy(vt[:, 0:256], qkv[:, 512:768])
                    nc.scalar.activation(
                        vt[:, 256:258], qkv[:, 512:514], AF.Identity,
                        bias=1.0, scale=0.0,
                    )

                    # context accumulation (heads packed in pairs on lhsT cols)
                    nc.tensor.matmul(
                        ctx01, lhsT=ek[:, 0:128], rhs=vt,
                        start=(t == 0), stop=(t == NSUB - 1), skip_group_check=True,
                    )
                    nc.tensor.matmul(
                        ctx23, lhsT=ek[:, 128:256], rhs=vt,
                        start=(t == 0), stop=(t == NSUB - 1), skip_group_check=True,
                    )

                    # transpose eqn to layout A and park in resident buffer
                    tr01 = trp.tile([C, C], F32R, tag="tr")
                    nc.tensor.transpose(tr01, eqn[:, 0:2, :], ident_r)
                    tr23 = trp.tile([C, C], F32R, tag="tr")
                    nc.tensor.transpose(tr23, eqn[:, 2:4, :], ident_r)
                    nc.scalar.copy(eqnA01[:, t * 128 : (t + 1) * 128], tr01)
                    nc.vector.tensor_copy(eqnA23[:, t * 128 : (t + 1) * 128], tr23)

                # ---- finalize context: divide by s_k, build block-diag lhsT ----
                lhsT01 = consts.tile([C, C], F32R)
                lhsT23 = consts.tile([C, C], F32R)
                r01 = small.tile([C, 1], F32, tag="r01")
                r23 = small.tile([C, 1], F32, tag="r23")
                nc.vector.reciprocal(r01, ctx01[:, 256:257])
                nc.vector.reciprocal(r23, ctx23[:, 256:257])
                nc.vector.tensor_scalar_mul(
                    lhsT01[0:64, 0:64], ctx01[0:64, 0:64], r01[0:64, 0:1]
                )
                nc.vector.tensor_scalar_mul(
                    lhsT01[64:128, 64:128], ctx01[64:128, 64:128], r01[64:128, 0:1]
                )
                nc.vector.tensor_scalar_mul(lhsT01[0:64, 64:128], ctx01[0:64, 64:128], 0.0)
                nc.vector.tensor_scalar_mul(lhsT01[64:128, 0:64], ctx01[64:128, 0:64], 0.0)
                nc.vector.tensor_scalar_mul(
                    lhsT23[0:64, 0:64], ctx23[0:64, 128:192], r23[0:64, 0:1]
                )
                nc.vector.tensor_scalar_mul(
                    lhsT23[64:128, 64:128], ctx23[64:128, 192:256], r23[64:128, 0:1]
                )
                nc.vector.tensor_scalar_mul(lhsT23[0:64, 64:128], ctx23[0:64, 0:64], 0.0)
                nc.vector.tensor_scalar_mul(lhsT23[64:128, 0:64], ctx23[64:128, 0:64], 0.0)

            # ---- pass 2 ----
            with (
                tc.tile_pool(name="outp", bufs=2, space="PSUM") as outp,
                tc.tile_pool(name="finp", bufs=2, space="PSUM") as finp,
            ):
                for blk in range(NBLK):
                    nsl = slice(blk * NB, (blk + 1) * NB)
                    o01 = outp.tile([C, NB], F32, tag="o01")
                    o23 = outp.tile([C, NB], F32, tag="o23")
                    nc.tensor.matmul(
                        o01, lhsT=lhsT01, rhs=eqnA01[:, nsl],
                        start=True, stop=True, skip_group_check=True,
                    )
                    nc.tensor.matmul(
                        o23, lhsT=lhsT23, rhs=eqnA23[:, nsl],
                        start=True, stop=True, skip_group_check=True,
                    )
                    att01 = work.tile([C, NB], F32R, tag="att01")
                    att23 = work.tile([C, NB], F32R, tag="att23")
                    nc.vector.tensor_copy(att01, o01)
                    nc.scalar.copy(att23, o23)
                    fin = finp.tile([C, NB], F32, tag="fin")
                    nc.tensor.matmul(
                        fin, lhsT=wo_s[:, 0, :], rhs=att01,
                        start=True, stop=False, skip_group_check=True,
                    )
                    nc.tensor.matmul(
                        fin, lhsT=wo_s[:, 1, :], rhs=att23,
                        start=False, stop=True, skip_group_check=True,
                    )
                    osb = work.tile([C, NB], F32, tag="osb")
                    nc.scalar.activation(
                        osb, fin, AF.Identity, bias=bo_s[:, 0:1], scale=1.0
                    )
                    nc.sync.dma_start(out=out[:, nsl], in_=osb)

    nc.compile()
    return nc


_NC_CACHE = None


def kernel(x, w_qkv, w_out, b_out):
    global _NC_CACHE
    if _NC_CACHE is None:
        _NC_CACHE = build_nc()
    nc = _NC_CACHE

    b = x.shape[0]
    wqT = np.ascontiguousarray(np.asarray(w_qkv, dtype=np.float32).T)      # [128, 768]
    woT = np.ascontiguousarray(np.asarray(w_out, dtype=np.float32).T)      # [256, 128]
    bo = np.ascontiguousarray(np.asarray(b_out, dtype=np.float32).reshape(C, 1))
    in_maps = [
        {
            "x": np.ascontiguousarray(np.asarray(x[i], dtype=np.float32).reshape(C, N)),
            "wqT": wqT,
            "woT": woT,
            "bo": bo,
        }
        for i in range(b)
    ]
    res = run_bass_kernel_spmd(nc, in_maps, core_ids=list(range(b)))
    out = np.stack(
        [res.results[i]["out"].reshape(C, 128, 128) for i in range(b)]
    ).astype(np.float32)
    return out
